# revision 1
# baseline (speedup 1.0000x reference)
"""Trainium2 Bass kernel for AttentionFusion (B=4, T=4, H=W=32, C=128).

Sharding: 8 cores = batch (4) x query-half (2). Each core computes full
attention for 2048 query rows of one batch element against all 4096 keys
of that element. No cross-core communication.

Per-core pipeline (raw Bass, manual semaphores; this walrus build allows
only one sync-wait per instruction, so waits are standalone wait_ge):
  PE : K/Q/V projections, S^T = K_tile^T @ Q_block (fp32r), rowsum via
       ones-matmul, O += V_tile @ P_tile (bf16), Wo projection, recip
       broadcast.
  ACT: exp(scale*S) PSUM->SBUF (bf16), one op per two 512-col PSUM banks.
  DVE: PSUM->SBUF copies (+bias adds), reciprocal, final normalize.
  POOL: DMA.
"""
import sys

sys.path.insert(0, "/opt/trn_rl_repo")

import numpy as np
import ml_dtypes

import concourse.bass as bass
import concourse.mybir as mybir
from concourse.bass_utils import run_bass_kernel_spmd

f32 = mybir.dt.float32
f32r = mybir.dt.float32r
bf16 = mybir.dt.bfloat16

B, T, C, H, W = 4, 4, 128, 32, 32
N = T * H * W            # 4096 keys per batch element
NLOC = N // 2            # 2048 query rows per core
NB = NLOC // 512         # 4 column blocks of 512 queries
MT = N // 128            # 32 key tiles
NG = MT // 2             # 16 exp groups per block (2 tiles each)
SCALE = float(C) ** -0.5

N_CORES = 8


def _build(stage="full"):
    nc = bass.Bass("TRN2")

    xs = nc.declare_dram_parameter("xs", [C, NLOC], f32r, isOutput=False)
    xt = nc.declare_dram_parameter("xt", [C, N], f32r, isOutput=False)
    xtb = nc.declare_dram_parameter("xtb", [C, N], bf16, isOutput=False)
    w3 = nc.declare_dram_parameter("w3", [C, 3 * C], f32r, isOutput=False)  # wqT|wkT|woT
    b3 = nc.declare_dram_parameter("b3", [C, 3], f32, isOutput=False)       # bq|bk|bo_eff
    onesr = nc.declare_dram_parameter("onesr", [1, C], f32, isOutput=False)
    wvob = nc.declare_dram_parameter("wvob", [C, C + 1], bf16, isOutput=False)  # ones_col|wvT
    out = nc.declare_dram_parameter("out", [C, NLOC], f32, isOutput=True)

    N_IN_DMAS = 7

    # ---- precomputed semaphore schedules (must mirror emission order) ----
    pe = 0
    k_mm, vt_mm, q_mm = {}, {}, {}
    for j in range(8):
        pe += 1; k_mm[j] = pe
    for mt in range(MT):
        pe += 1; vt_mm[mt] = pe
    for j in range(4):
        pe += 1; q_mm[j] = pe
    st_cnt, rs_cnt, pv_cnt, rb_cnt, y_cnt = {}, {}, {}, {}, {}
    for nb in range(NB):
        for g in range(NG):
            pe += 1; st_cnt[(nb, 2 * g)] = pe
            pe += 1; st_cnt[(nb, 2 * g + 1)] = pe
            if g >= 1:
                for k in (2 * g - 2, 2 * g - 1):
                    pe += 1; rs_cnt[(nb, k)] = pe
                    pe += 1; pv_cnt[(nb, k)] = pe
        for k in (MT - 2, MT - 1):
            pe += 1; rs_cnt[(nb, k)] = pe
            pe += 1; pv_cnt[(nb, k)] = pe
        pe += 1; rb_cnt[nb] = pe
        pe += 1; y_cnt[nb] = pe

    dve = 0
    kcopy, vtcopy, qcopy = {}, {}, {}
    for j in range(8):
        dve += 1; kcopy[j] = dve
    for mt in range(MT):
        dve += 1; vtcopy[mt] = dve
    for j in range(4):
        dve += 1; qcopy[j] = dve
    PROJ_DVE = dve  # 44
    rs_free, rcr_ready, o_ready, rb_ready, y_ready = {}, {}, {}, {}, {}
    for nb in range(NB):
        dve += 1; rs_free[nb] = dve    # reciprocal (reads rs_ps)
        rcr_ready[nb] = dve            # recip result is consumed directly
        dve += 1; o_ready[nb] = dve    # O copy
        dve += 1; rb_ready[nb] = dve   # rb copy
        dve += 1                       # mul
        dve += 1; y_ready[nb] = dve    # bias add -> Y block done

    from contextlib import ExitStack
    ctx = ExitStack()
    with ctx:
        def sb(name, shape, dt):
            return ctx.enter_context(nc.sbuf_tensor(name, shape, dt))
        def ps(name, shape, dt):
            return ctx.enter_context(nc.psum_tensor(name, shape, dt))
        s_xs = sb("s_xs", [C, NLOC], f32r)
        s_xt = sb("s_xt", [C, N], f32r)
        s_xtb = sb("s_xtb", [C, N], bf16)
        s_w3 = sb("s_w3", [C, 3 * C], f32r)
        s_b3 = sb("s_b3", [C, 3], f32)
        s_onesr = sb("s_onesr", [1, C], f32)
        s_wvob = sb("s_wvob", [C, C + 1], bf16)
        s_K = sb("s_K", [C, N], f32r)
        s_Q = sb("s_Q", [C, NLOC], f32r)
        s_VT = sb("s_VT", [C, N], bf16)          # 32 tiles of [128,128]
        s_PT = sb("s_PT", [C, 8 * 512], bf16)    # 8 ring slots of [128,512]
        s_O = sb("s_O", [C, 512], f32r)
        s_rc = sb("s_rc", [1, 512], f32)
        s_rb = sb("s_rb", [C, 512], f32)
        s_ytmp = sb("s_ytmp", [C, 512], f32)
        s_Y = sb("s_Y", [C, NLOC], f32)
        st_ps0 = ps("st_ps0", [C, 1024], f32)
        st_ps1 = ps("st_ps1", [C, 1024], f32)
        o_ps = ps("o_ps", [C, 512], f32)
        rs_ps = ps("rs_ps", [1, 512], f32)
        rb_ps = ps("rb_ps", [C, 512], f32)
        y_ps = ps("y_ps", [C, 512], f32)
        dma_sem = ctx.enter_context(nc.semaphore("dma_sem"))
        pe_sem = ctx.enter_context(nc.semaphore("pe_sem"))
        act_sem = ctx.enter_context(nc.semaphore("act_sem"))
        dve_sem = ctx.enter_context(nc.semaphore("dve_sem"))
        block = ctx.enter_context(nc.Block())

        st_ps = [st_ps0, st_ps1]
        vt_slots = [rb_ps, y_ps]  # VT projection scratch: 8 slots of [128,128]

        def st_slot(j):
            # 4 rotating [128,512] psum slots used by K/Q proj and main ST
            return st_ps[(j // 2) % 2][:, (j % 2) * 512:(j % 2) * 512 + 512]

        def pt_slot(kglob, ntiles=1):
            s = kglob % 8
            return s_PT[:, s * 512:(s + ntiles) * 512]

        @block.tensor
        def _(tensor):
            tensor.wait_ge(dma_sem, 16 * N_IN_DMAS)
            # K projection: K[c, m] = wkT.T @ xt
            for j in range(8):
                if j >= 4:
                    tensor.wait_ge(dve_sem, kcopy[j - 4])
                nc.tensor.matmul(st_slot(j), s_w3[:, C:2 * C], s_xt[:, j * 512:(j + 1) * 512],
                                 start=True, stop=True).then_inc(pe_sem, 1)
            # V^T tiles: VT[m, c] = xt_tile.T @ wvT   (bf16)
            for mt in range(MT):
                if mt >= 2:
                    tensor.wait_ge(dve_sem, vtcopy[mt - 2])
                slot = vt_slots[mt % 2][:, ((mt // 2) % 2) * 128:((mt // 2) % 2) * 128 + 128]
                nc.tensor.matmul(slot, s_xtb[:, mt * 128:(mt + 1) * 128], s_wvob[:, 1:],
                                 start=True, stop=True).then_inc(pe_sem, 1)
            # Q projection
            for j in range(4):
                tensor.wait_ge(dve_sem, kcopy[4 + j])
                nc.tensor.matmul(st_slot(8 + j), s_w3[:, 0:C], s_xs[:, j * 512:(j + 1) * 512],
                                 start=True, stop=True).then_inc(pe_sem, 1)

            def rs_pv(nb, k):
                pt = pt_slot(nb * MT + k)
                nc.tensor.matmul(rs_ps[:], s_wvob[:, 0:1], pt,
                                 start=(k == 0), stop=(k == MT - 1)).then_inc(pe_sem, 1)
                nc.tensor.matmul(o_ps[:], s_VT[:, k * 128:(k + 1) * 128], pt,
                                 start=(k == 0), stop=(k == MT - 1)).then_inc(pe_sem, 1)

            for nb in range(NB):
                q_rhs = s_Q[:, nb * 512:(nb + 1) * 512]
                for g in range(NG):
                    if g == 0:
                        if nb == 0:
                            tensor.wait_ge(dve_sem, PROJ_DVE)  # proj copies done
                        else:
                            tensor.wait_ge(act_sem, nb * NG)   # prev block exps done
                    for mt in (2 * g, 2 * g + 1):
                        nc.tensor.matmul(st_slot(mt), s_K[:, mt * 128:(mt + 1) * 128],
                                         q_rhs, start=True, stop=True).then_inc(pe_sem, 1)
                    if g >= 1:
                        tensor.wait_ge(act_sem, nb * NG + g)   # exp group g-1 done
                        if g == 1 and nb >= 1:
                            tensor.wait_ge(dve_sem, rs_free[nb - 1])  # rs_ps free
                            tensor.wait_ge(dve_sem, o_ready[nb - 1])  # o_ps free
                        rs_pv(nb, 2 * g - 2)
                        rs_pv(nb, 2 * g - 1)
                tensor.wait_ge(act_sem, nb * NG + NG)
                rs_pv(nb, MT - 2)
                rs_pv(nb, MT - 1)
                # epilogue matmuls
                tensor.wait_ge(dve_sem, rcr_ready[nb])
                nc.tensor.matmul(rb_ps[:], s_onesr[:], s_rc[:],
                                 start=True, stop=True).then_inc(pe_sem, 1)
                tensor.wait_ge(dve_sem, o_ready[nb])
                nc.tensor.matmul(y_ps[:], s_w3[:, 2 * C:3 * C], s_O[:],
                                 start=True, stop=True).then_inc(pe_sem, 1)

        @block.scalar
        def _(scalar):
            for nb in range(NB):
                for g in range(NG):
                    kglob = nb * MT + 2 * g
                    scalar.wait_ge(pe_sem, st_cnt[(nb, 2 * g + 1)])
                    if kglob >= 8:
                        # PT ring slots freed once pv of tile kglob-7 issued
                        prev = kglob - 7
                        scalar.wait_ge(pe_sem, pv_cnt[(prev // MT, prev % MT)])
                    nc.scalar.activation(pt_slot(kglob, 2), st_ps[g % 2][:],
                                         mybir.ActivationFunctionType.Exp,
                                         scale=SCALE).then_inc(act_sem, 1)

        @block.vector
        def _(vector):
            # projection copies
            for j in range(8):
                vector.wait_ge(pe_sem, k_mm[j])
                vector.tensor_scalar_add(s_K[:, j * 512:(j + 1) * 512], st_slot(j),
                                         s_b3[:, 1:2]).then_inc(dve_sem, 1)
            for mt in range(MT):
                vector.wait_ge(pe_sem, vt_mm[mt])
                slot = vt_slots[mt % 2][:, ((mt // 2) % 2) * 128:((mt // 2) % 2) * 128 + 128]
                vector.tensor_copy(s_VT[:, mt * 128:(mt + 1) * 128], slot).then_inc(dve_sem, 1)
            for j in range(4):
                vector.wait_ge(pe_sem, q_mm[j])
                vector.tensor_scalar_add(s_Q[:, j * 512:(j + 1) * 512], st_slot(8 + j),
                                         s_b3[:, 0:1]).then_inc(dve_sem, 1)
            # per-block epilogue (drains between same-engine dependent ops)
            for nb in range(NB):
                vector.wait_ge(pe_sem, rs_cnt[(nb, MT - 1)])
                vector.reciprocal(s_rc[:], rs_ps[:]).then_inc(dve_sem, 1)
                vector.wait_ge(pe_sem, pv_cnt[(nb, MT - 1)])
                vector.tensor_copy(s_O[:], o_ps[:]).then_inc(dve_sem, 1)
                vector.wait_ge(pe_sem, rb_cnt[nb])
                vector.tensor_copy(s_rb[:], rb_ps[:]).then_inc(dve_sem, 1)
                vector.wait_ge(pe_sem, y_cnt[nb])
                vector.drain()
                vector.tensor_mul(s_ytmp[:], y_ps[:], s_rb[:]).then_inc(dve_sem, 1)
                vector.drain()
                vector.tensor_scalar_add(s_Y[:, nb * 512:(nb + 1) * 512], s_ytmp[:],
                                         s_b3[:, 2:3]).then_inc(dve_sem, 1)

        @block.gpsimd
        def _(gpsimd):
            gpsimd.dma_start(s_xt[:], xt[:]).then_inc(dma_sem, 16)
            gpsimd.dma_start(s_xtb[:], xtb[:]).then_inc(dma_sem, 16)
            gpsimd.dma_start(s_xs[:], xs[:]).then_inc(dma_sem, 16)
            gpsimd.dma_start(s_w3[:], w3[:]).then_inc(dma_sem, 16)
            gpsimd.dma_start(s_b3[:], b3[:]).then_inc(dma_sem, 16)
            gpsimd.dma_start(s_onesr[:], onesr[:]).then_inc(dma_sem, 16)
            gpsimd.dma_start(s_wvob[:], wvob[:]).then_inc(dma_sem, 16)
            for nb in range(NB):
                gpsimd.wait_ge(dve_sem, y_ready[nb])
                gpsimd.dma_start(out[:, nb * 512:(nb + 1) * 512],
                                 s_Y[:, nb * 512:(nb + 1) * 512]).then_inc(dma_sem, 16)

    return nc


def _make_in_maps(spatial_features, temporal_features, Wq, bq, Wk, bk, Wv, bv, Wo, bo):
    f = np.float32
    bf = ml_dtypes.bfloat16
    w3 = np.ascontiguousarray(np.concatenate([Wq.T, Wk.T, Wo.T], axis=1)).astype(f)
    bo_eff = (Wo @ bv + bo).astype(f)
    b3 = np.ascontiguousarray(np.stack([bq, bk, bo_eff], axis=1)).astype(f)  # [C, 3]
    onesr = np.ones((1, C), f)
    wvob = np.ascontiguousarray(
        np.concatenate([np.ones((C, 1), f), Wv.T], axis=1)).astype(bf)

    in_maps = []
    for core in range(N_CORES):
        b, half = core // 2, core % 2
        xs_ = np.ascontiguousarray(
            spatial_features[b, 2 * half:2 * half + 2]      # [2, C, H, W]
            .transpose(1, 0, 2, 3).reshape(C, NLOC)).astype(f)
        xt_ = np.ascontiguousarray(temporal_features[b].reshape(C, N)).astype(f)
        in_maps.append({
            "xs": xs_,
            "xt": xt_,
            "xtb": xt_.astype(bf),
            "w3": w3,
            "b3": b3,
            "onesr": onesr,
            "wvob": wvob,
        })
    return in_maps


_CACHED = {}


def _run(in_maps, trace=False):
    import os
    stage = os.environ.get("KSTAGE", "full")
    if _CACHED.get("stage") != stage:
        _CACHED["nc"] = _build(stage)
        _CACHED["stage"] = stage
    return run_bass_kernel_spmd(_CACHED["nc"], in_maps, list(range(N_CORES)), trace=trace)


def kernel(spatial_features, temporal_features, Wq, bq, Wk, bk, Wv, bv, Wo, bo):
    args = [np.asarray(a) for a in (spatial_features, temporal_features,
                                    Wq, bq, Wk, bk, Wv, bv, Wo, bo)]
    in_maps = _make_in_maps(*args)
    res = _run(in_maps)
    out = np.empty((B, C, T, H, W), np.float32)
    for core in range(N_CORES):
        b, half = core // 2, core % 2
        y = res.results[core]["out"]                        # [C, NLOC]
        out[b, :, 2 * half:2 * half + 2] = np.asarray(y).reshape(C, 2, H, W)
    return out



# revision 12
# speedup vs baseline: 1.0444x; 1.0444x over previous
"""Trainium2 Bass kernel for AttentionFusion (B=4, T=4, H=W=32, C=128).

Sharding: 8 cores = batch (4) x query-half (2). Each core computes full
attention for 2048 query rows of one batch element against all 4096 keys.

v2 design (vs fp32/rowsum-matmul baseline):
  - fp16 for all matmul operands (ST, PV, projections).
  - Rowsum no longer burns a 512-col PE stream per key tile: DVE builds
    oct-sums (tree of 7 fp16 adds per 8 tiles), PE does 4 rowsum matmuls
    per block instead of 32.
  - exp split across engines: ACT does true exp for 12/16 pairs, DVE does
    4/16 pairs via a Schraudolph bit-hack (i16 = s*1024/ln2 + 15*1024-44,
    bitcast to fp16; ~3% max elementwise error, averages out in softmax).
  - VT psum->sbuf casts moved to ACT; K/Q bias-adds on DVE.

Engines: PE matmuls; ACT exp + VT casts; DVE schraudolph-exp, adds,
epilogue; gpsimd DMA. Raw Bass, standalone wait_ge, precomputed counters.
"""
import math
import sys

sys.path.insert(0, "/opt/trn_rl_repo")

import numpy as np
import ml_dtypes

import concourse.bass as bass
import concourse.mybir as mybir
from concourse.bass_utils import run_bass_kernel_spmd

f32 = mybir.dt.float32
f16 = mybir.dt.float16
i16 = mybir.dt.int16

B, T, C, H, W = 4, 4, 128, 32, 32
N = T * H * W            # 4096 keys per batch element
NLOC = N // 2            # 2048 query rows per core
NB = NLOC // 512         # 4 column blocks of 512 queries
MT = N // 128            # 32 key tiles
PAIRS = MT // 2          # 16 tile-pairs per block
GTOT = NB * PAIRS        # 64 global pairs
OCTS = PAIRS // 4        # 4 octs (8 tiles) per block
SCALE = float(C) ** -0.5

import os
DVE_RES = tuple(int(x) for x in os.environ.get("V_DVE_RES", "3,7").split(",") if x != "")
A16 = SCALE * 1024.0 / math.log(2.0)
B16 = 15.0 * 1024.0 - 44.0

N_CORES = 8


def _dve_exp(G):
    return (G % 8) in DVE_RES


def _sched():
    """Precompute semaphore counter values (mirrors emission order)."""
    s = {}
    pe = 0
    k_mm, vt_mm, q_mm = {}, {}, {}
    st_done, pv_done, rsmm, rb_mm, y_mm = {}, {}, {}, {}, {}
    for j in range(8):
        pe += 1; k_mm[j] = pe
    for m in range(MT):
        pe += 1; vt_mm[m] = pe
    for j in range(4):
        pe += 1; q_mm[j] = pe
    for nb in range(NB):
        for g in range(PAIRS):
            G = nb * PAIRS + g
            pe += 1; st_done[G] = pe
            if g >= 1:
                pe += 1; pv_done[G - 1] = pe
            if g >= 4 and g % 4 == 0:
                pe += 1; rsmm[(nb, g // 4 - 1)] = pe
        pe += 1; pv_done[nb * PAIRS + PAIRS - 1] = pe
        pe += 1; rsmm[(nb, 3)] = pe
        pe += 1; rb_mm[nb] = pe
        pe += 1; y_mm[nb] = pe

    act = 0
    vt_cast, exp_act = {}, {}
    for m in range(MT):
        act += 1; vt_cast[m] = act
    for G in range(GTOT):
        if not _dve_exp(G):
            act += 1; exp_act[G] = act

    dve = 0
    k_copy, q_copy = {}, {}
    exp_dve, a_done, q_done, oct_done, recip, rbcopy, omul, yadd = {}, {}, {}, {}, {}, {}, {}, {}
    for j in range(8):
        dve += 1; k_copy[j] = dve
    for j in range(4):
        dve += 1; q_copy[j] = dve
    for nb in range(NB):
        for g in range(PAIRS):
            G = nb * PAIRS + g
            if _dve_exp(G):
                dve += 1; exp_dve[G] = dve
            dve += 1; a_done[G] = dve          # pair add
            if g % 4 == 3:
                o = g // 4
                dve += 1; q_done[(nb, 2 * o)] = dve
                dve += 1; q_done[(nb, 2 * o + 1)] = dve
                dve += 1; oct_done[(nb, o)] = dve
        dve += 1; recip[nb] = dve
        dve += 1; rbcopy[nb] = dve
        dve += 1; omul[nb] = dve
        dve += 1; yadd[nb] = dve

    s.update(k_mm=k_mm, vt_mm=vt_mm, q_mm=q_mm, st_done=st_done,
             pv_done=pv_done, rsmm=rsmm, rb_mm=rb_mm, y_mm=y_mm,
             vt_cast=vt_cast, exp_act=exp_act, k_copy=k_copy, q_copy=q_copy,
             exp_dve=exp_dve, a_done=a_done, q_done=q_done, oct_done=oct_done, recip=recip,
             rbcopy=rbcopy, omul=omul, yadd=yadd)
    return s


def _build():
    nc = bass.Bass("TRN2")
    S = _sched()

    xs = nc.declare_dram_parameter("xs", [C, NLOC], f16, isOutput=False)
    xt = nc.declare_dram_parameter("xt", [C, N], f16, isOutput=False)
    w3 = nc.declare_dram_parameter("w3", [C, 3 * C], f16, isOutput=False)  # wqT|wkT|woT
    wv = nc.declare_dram_parameter("wv", [C, C], f16, isOutput=False)      # wvT
    b3 = nc.declare_dram_parameter("b3", [C, 3], f32, isOutput=False)      # bq|bk|bo_eff
    out = nc.declare_dram_parameter("out", [C, NLOC], f32, isOutput=True)

    from contextlib import ExitStack
    ctx = ExitStack()
    with ctx:
        def sb(name, shape, dt):
            return ctx.enter_context(nc.sbuf_tensor(name, shape, dt))
        def ps(name, shape, dt):
            return ctx.enter_context(nc.psum_tensor(name, shape, dt))
        s_xs = sb("s_xs", [C, NLOC], f16)
        s_xt = sb("s_xt", [C, N], f16)
        s_w3 = sb("s_w3", [C, 3 * C], f16)
        s_wv = sb("s_wv", [C, C], f16)
        s_b3 = sb("s_b3", [C, 3], f32)
        s_ones1 = sb("s_ones1", [1, C], f16)    # rb broadcast lhsT
        s_onesc = sb("s_onesc", [C, 1], f16)    # rowsum lhsT
        s_K = sb("s_K", [C, N], f16)
        s_Q = sb("s_Q", [C, NLOC], f16)
        s_VT = sb("s_VT", [C, MT, 128], f16)
        s_PT = sb("s_PT", [C, 8, 512], f16)     # 8 ring slots = 4 pairs
        s_ACC = sb("s_ACC", [C, 6, 512], f16)   # 4 pair + 2 quad partials
        s_OCT = sb("s_OCT", [C, 2, 512], f16)   # oct sums (double buffered)
        s_O = sb("s_O", [C, 512], f16)
        s_rc = sb("s_rc", [1, 512], f16)
        s_rb = sb("s_rb", [C, 512], f16)
        s_Y = sb("s_Y", [C, NLOC], f32)
        st_ps0 = ps("st_ps0", [C, 2, 512], f32)
        st_ps1 = ps("st_ps1", [C, 2, 512], f32)
        o_ps = ps("o_ps", [C, 512], f32)
        rs_ps = ps("rs_ps", [1, 512], f32)
        rb_ps = ps("rb_ps", [C, 512], f32)
        y_ps = ps("y_ps", [C, 512], f32)
        dma_kw = ctx.enter_context(nc.semaphore("dma_kw"))    # xt + w3
        dma_v = ctx.enter_context(nc.semaphore("dma_v"))      # wv
        dma_b = ctx.enter_context(nc.semaphore("dma_b"))      # b3
        dma_x = ctx.enter_context(nc.semaphore("dma_x"))      # xs
        dma_out = ctx.enter_context(nc.semaphore("dma_out"))
        pe_sem = ctx.enter_context(nc.semaphore("pe_sem"))
        act_sem = ctx.enter_context(nc.semaphore("act_sem"))
        dve_sem = ctx.enter_context(nc.semaphore("dve_sem"))
        block = ctx.enter_context(nc.Block())

        st_ps = [st_ps0, st_ps1]

        def st_slot(i):
            # 4 rotating [128,512] psum slots (proj phase)
            return st_ps[(i // 2) % 2][:, i % 2, :]

        def vt_slot(m):
            return (rb_ps if (m // 4) % 2 == 0 else y_ps)[:, (m % 4) * 128:(m % 4 + 1) * 128]

        def pt_pair(G):
            sl = (2 * G) % 8
            return s_PT[:, sl:sl + 2, :]

        def pt_tile(t):
            return s_PT[:, t % 8, :]

        @block.tensor
        def _(tensor):
            # K projection: K[c,m] = wkT.T @ xt
            tensor.wait_ge(dma_kw, 16 * 2)
            for j in range(8):
                if j >= 4:
                    tensor.wait_ge(dve_sem, S["k_copy"][j - 4])
                nc.tensor.matmul(st_slot(j), s_w3[:, C:2 * C], s_xt[:, j * 512:(j + 1) * 512],
                                 start=True, stop=True).then_inc(pe_sem, 1)
            # V^T tiles: VT[m,c] = xt_tile.T @ wvT
            tensor.wait_ge(dma_v, 16)
            for m in range(MT):
                if m >= 8:
                    tensor.wait_ge(act_sem, S["vt_cast"][m - 8])
                nc.tensor.matmul(vt_slot(m), s_xt[:, m * 128:(m + 1) * 128], s_wv[:],
                                 start=True, stop=True).then_inc(pe_sem, 1)
            # Q projection
            tensor.wait_ge(dma_x, 16)
            for j in range(4):
                tensor.wait_ge(dve_sem, S["k_copy"][4 + j])
                nc.tensor.matmul(st_slot(8 + j), s_w3[:, 0:C], s_xs[:, j * 512:(j + 1) * 512],
                                 start=True, stop=True).then_inc(pe_sem, 1)

            def emit_pv(P):
                nb, p = P // PAIRS, P % PAIRS
                if _dve_exp(P):
                    tensor.wait_ge(dve_sem, S["exp_dve"][P])
                    if nb == 0:
                        tensor.wait_ge(act_sem, S["vt_cast"][2 * p + 1])
                else:
                    tensor.wait_ge(act_sem, S["exp_act"][P])
                if p == 0 and nb >= 1:
                    tensor.wait_ge(dve_sem, S["omul"][nb - 1])
                for t in (2 * P, 2 * P + 1):
                    mm = nc.tensor.matmul(o_ps[:], s_VT[:, t % MT, :], pt_tile(t),
                                          start=(t % MT == 0), stop=(t % MT == MT - 1))
                mm.then_inc(pe_sem, 1)

            def emit_rs(nb, o):
                tensor.wait_ge(dve_sem, S["oct_done"][(nb, o)])
                if o == 0 and nb >= 1:
                    tensor.wait_ge(dve_sem, S["recip"][nb - 1])
                nc.tensor.matmul(rs_ps[:], s_onesc[:], s_OCT[:, o % 2, :],
                                 start=(o == 0), stop=(o == 3)).then_inc(pe_sem, 1)

            for nb in range(NB):
                q_rhs = s_Q[:, nb * 512:(nb + 1) * 512]
                for g in range(PAIRS):
                    G = nb * PAIRS + g
                    # ST pair -> st_ps[G%2]
                    if G < 2:
                        tensor.wait_ge(dve_sem, S["q_copy"][2 * G + 1])
                    else:
                        Gp = G - 2
                        if _dve_exp(Gp):
                            tensor.wait_ge(dve_sem, S["exp_dve"][Gp])
                        else:
                            tensor.wait_ge(act_sem, S["exp_act"][Gp])
                    for h in range(2):
                        t = 2 * G + h
                        mm = nc.tensor.matmul(st_ps[G % 2][:, h, :],
                                              s_K[:, (t % MT) * 128:(t % MT) * 128 + 128],
                                              q_rhs, start=True, stop=True)
                    mm.then_inc(pe_sem, 1)
                    if g >= 1:
                        emit_pv(G - 1)
                    if g >= 4 and g % 4 == 0:
                        emit_rs(nb, g // 4 - 1)
                emit_pv(nb * PAIRS + PAIRS - 1)
                emit_rs(nb, 3)
                # epilogue matmuls
                tensor.wait_ge(dve_sem, S["recip"][nb])
                if nb == 0:
                    tensor.wait_ge(act_sem, S["vt_cast"][MT - 1])
                nc.tensor.matmul(rb_ps[:], s_ones1[:], s_rc[:],
                                 start=True, stop=True).then_inc(pe_sem, 1)
                tensor.wait_ge(dve_sem, S["omul"][nb])
                nc.tensor.matmul(y_ps[:], s_w3[:, 2 * C:3 * C], s_O[:],
                                 start=True, stop=True).then_inc(pe_sem, 1)

        @block.scalar
        def _(scalar):
            # VT psum -> sbuf casts
            for m in range(MT):
                scalar.wait_ge(pe_sem, S["vt_mm"][m])
                nc.scalar.copy(s_VT[:, m, :], vt_slot(m)).then_inc(act_sem, 1)
            # true exp for ACT-owned pairs
            for G in range(GTOT):
                if _dve_exp(G):
                    continue
                scalar.wait_ge(pe_sem, S["st_done"][G])
                if G >= 4:
                    scalar.wait_ge(pe_sem, S["pv_done"][G - 4])
                    scalar.wait_ge(dve_sem, S["a_done"][G - 4])
                nc.scalar.activation(pt_pair(G), st_ps[G % 2][:],
                                     mybir.ActivationFunctionType.Exp,
                                     scale=SCALE).then_inc(act_sem, 1)

        @block.vector
        def _(vector):
            nc.vector.memset(s_ones1[:], 1.0)
            nc.vector.memset(s_onesc[:], 1.0)
            vector.wait_ge(dma_b, 16)
            for j in range(8):
                vector.wait_ge(pe_sem, S["k_mm"][j])
                nc.vector.tensor_scalar_add(s_K[:, j * 512:(j + 1) * 512], st_slot(j),
                                            s_b3[:, 1:2]).then_inc(dve_sem, 1)
            for j in range(4):
                vector.wait_ge(pe_sem, S["q_mm"][j])
                nc.vector.tensor_scalar_add(s_Q[:, j * 512:(j + 1) * 512], st_slot(8 + j),
                                            s_b3[:, 0:1]).then_inc(dve_sem, 1)
            for nb in range(NB):
                for g in range(PAIRS):
                    G = nb * PAIRS + g
                    if _dve_exp(G):
                        vector.wait_ge(pe_sem, S["st_done"][G])
                        if G >= 4:
                            vector.wait_ge(pe_sem, S["pv_done"][G - 4])
                        nc.vector.tensor_scalar(
                            pt_pair(G).bitcast(i16), st_ps[G % 2][:],
                            A16, B16, mybir.AluOpType.mult,
                            mybir.AluOpType.add).then_inc(dve_sem, 1)
                        vector.drain()
                    else:
                        vector.wait_ge(act_sem, S["exp_act"][G])
                    # pair add: A(g) = pt(2G) + pt(2G+1) -> ACC[g%4]
                    nc.vector.tensor_add(s_ACC[:, g % 4, :], pt_tile(2 * G),
                                         pt_tile(2 * G + 1)).then_inc(dve_sem, 1)
                    if g % 4 == 3:
                        o = g // 4
                        vector.drain()
                        nc.vector.tensor_add(s_ACC[:, 4, :], s_ACC[:, 0, :],
                                             s_ACC[:, 1, :]).then_inc(dve_sem, 1)
                        nc.vector.tensor_add(s_ACC[:, 5, :], s_ACC[:, 2, :],
                                             s_ACC[:, 3, :]).then_inc(dve_sem, 1)
                        po, pnb = (o - 2, nb) if o >= 2 else (o + 2, nb - 1)
                        if pnb >= 0:
                            vector.wait_ge(pe_sem, S["rsmm"][(pnb, po)])
                        vector.drain()
                        nc.vector.tensor_add(s_OCT[:, o % 2, :], s_ACC[:, 4, :],
                                             s_ACC[:, 5, :]).then_inc(dve_sem, 1)
                # epilogue
                vector.wait_ge(pe_sem, S["rsmm"][(nb, 3)])
                with nc.allow_low_precision(reason="fp16 reciprocal of rowsum"):
                    nc.vector.reciprocal(s_rc[:], rs_ps[:]).then_inc(dve_sem, 1)
                vector.wait_ge(pe_sem, S["rb_mm"][nb])
                nc.vector.tensor_copy(s_rb[:], rb_ps[:]).then_inc(dve_sem, 1)
                vector.wait_ge(pe_sem, S["pv_done"][nb * PAIRS + PAIRS - 1])
                vector.drain()
                nc.vector.tensor_mul(s_O[:], o_ps[:], s_rb[:]).then_inc(dve_sem, 1)
                vector.wait_ge(pe_sem, S["y_mm"][nb])
                nc.vector.tensor_scalar_add(s_Y[:, nb * 512:(nb + 1) * 512], y_ps[:],
                                            s_b3[:, 2:3]).then_inc(dve_sem, 1)

        @block.gpsimd
        def _(gpsimd):
            gpsimd.dma_start(s_xt[:], xt[:]).then_inc(dma_kw, 16)
            gpsimd.dma_start(s_w3[:], w3[:]).then_inc(dma_kw, 16)
            gpsimd.dma_start(s_wv[:], wv[:]).then_inc(dma_v, 16)
            gpsimd.dma_start(s_b3[:], b3[:]).then_inc(dma_b, 16)
            gpsimd.dma_start(s_xs[:], xs[:]).then_inc(dma_x, 16)
            for nb in range(NB):
                gpsimd.wait_ge(dve_sem, S["yadd"][nb])
                gpsimd.dma_start(out[:, nb * 512:(nb + 1) * 512],
                                 s_Y[:, nb * 512:(nb + 1) * 512]).then_inc(dma_out, 16)

    return nc


def _make_in_maps(spatial_features, temporal_features, Wq, bq, Wk, bk, Wv, bv, Wo, bo):
    f = np.float32
    h = np.float16
    w3 = np.ascontiguousarray(np.concatenate([Wq.T, Wk.T, Wo.T], axis=1)).astype(h)
    wv = np.ascontiguousarray(Wv.T).astype(h)
    bo_eff = (Wo @ bv + bo).astype(f)
    b3 = np.ascontiguousarray(np.stack([bq, bk, bo_eff], axis=1)).astype(f)

    in_maps = []
    for core in range(N_CORES):
        b, half = core // 2, core % 2
        xs_ = np.ascontiguousarray(
            spatial_features[b, 2 * half:2 * half + 2]
            .transpose(1, 0, 2, 3).reshape(C, NLOC)).astype(h)
        xt_ = np.ascontiguousarray(temporal_features[b].reshape(C, N)).astype(h)
        in_maps.append({"xs": xs_, "xt": xt_, "w3": w3, "wv": wv, "b3": b3})
    return in_maps


_CACHED = {}


def _run(in_maps, trace=False):
    if "nc" not in _CACHED:
        _CACHED["nc"] = _build()
    return run_bass_kernel_spmd(_CACHED["nc"], in_maps, list(range(N_CORES)), trace=trace)


def kernel(spatial_features, temporal_features, Wq, bq, Wk, bk, Wv, bv, Wo, bo):
    args = [np.asarray(a) for a in (spatial_features, temporal_features,
                                    Wq, bq, Wk, bk, Wv, bv, Wo, bo)]
    in_maps = _make_in_maps(*args)
    res = _run(in_maps)
    out = np.empty((B, C, T, H, W), np.float32)
    for core in range(N_CORES):
        b, half = core // 2, core % 2
        y = res.results[core]["out"]
        out[b, :, 2 * half:2 * half + 2] = np.asarray(y).reshape(C, 2, H, W)
    return out
